# revision 35
# baseline (speedup 1.0000x reference)
"""DynamicConv Trainium2 kernel.

Math (B=1, L=2048, D=128, E=128, F=8, K1=K2=3, M=K2*D=384):
  f   = u @ proj                                   [L, F]
  kp[l,e,m] = sum_{k1,fc} f_pad[l+k1-1,fc] * W[e,k1,fc,m] + b[e,m]
  out[l,e]  = sum_{d,k2} u_pad[l+k2-1,d] * kp[l,e,d*K2+k2]

Factorized as out[l,e] = sum_j f_tap[l,j] * A_j[l,e] + bias_t[l,e] with
A/bias/f all produced by 3 shifted bf16 matmuls per l-tile of 128 positions
accumulated in PSUM; proj columns are embedded in the rhs so f_tap falls out
of the same matmuls.  PSUM layout (25-wide blocks):
  e*25 + j  (j<24): A_j[l,e];  j=24: bias_t[l,e]
  400 + j   (j<24): f_tap[l,j]
The combine runs on a CUSTOM DVE op (registered at build time through the
documented dve_ops extension point): SEG_MUL_CUMSUM computes
  cum[p, k] = sum_{k'<=k} in0[p, k'] * in1[p, k']
in one 1x pass, reading the A/bias columns STRAIGHT FROM PSUM in fp32.
Per-(e) segment sums are boundary differences of the global cumsum:
  out[l, e] = cum[l, e*25+24] - cum[l, e*25-1]     (cum[-1] := 0)
a [128, q, 16] strided tensor_tensor subtract on the otherwise-idle
GpSimd engine (leaving Vector with only the scans, which then keep up
with the PE's group period).  in1 is "fone" =
f_tap replicated over the 16 e-blocks with 1.0 in the bias slot (j=24),
materialized per group by ONE broadcast ACT copy straight from the
PSUM f block.

v2 changes over the 27.0us baseline (trace-driven):
 - Output path: TT subtracts write a persistent [128, NT, EL] SBUF
   tensor; ONE output DMA (1KB/partition rows, ~250B/ns) is issued
   AFTER the TileContext exit barrier, so the tile epilogue no longer
   serializes on output-DMA completion (~2.4us of small-packet DMA
   drain now hides under the fixed ~6.8us walrus semaphore-wipe
   epilogue that ends every NEFF).
 - The PE stream must have NO idle gaps: the tensor engine boots with a
   ~50% utilization cap that only lifts after sustained continuous
   activity (and on a power-drained device the grant comes even later),
   so every pre-unlock stall is paid twice.  Warm-up matmuls abut the
   DMA-gated stream start, per-tile k-order follows w-block arrival
   (w0|w2|w1, KORD), and pair groups interleave sub-tiles so the
   last-arriving w1 gates only matmul 5 of 6.
 - fone/cum constant-block memsets run on the otherwise-idle Vector
   engine; gpsimd only carries the latest-needed u chunks (its SW-DGE
   queue is slower and dribbles tail descriptors).
 - Both trailing single-tile groups split their f/proj columns into a
   separate 3-matmul PSUM accumulation group BEFORE the A columns, so
   the fone ACT copies overlap A matmuls and the post-stream drain is
   just scan+subtract (~0.9us instead of ~2.4us).

E is sharded 8 ways (16 channels/core); u is replicated.
"""

import os

import numpy as np
import ml_dtypes

# Open the NRT session with freshly-reset cores: repeated NEFF executions
# leave the tensor engine's power-management state drained (its 100%-
# utilization grant then comes ~6us later and the matmul stream runs at
# half rate), and a core reset restores it.  No-op if the runtime is
# already initialized or the variable is already set.
os.environ.setdefault("NEURON_RT_RESET_CORES", "1")

BF16 = ml_dtypes.bfloat16

B, L, D = 1, 2048, 128
E, F = 128, 8
K1, K2 = 3, 3
M = K2 * D
NCORES = 8
EL = E // NCORES          # 16 output channels per core
NJ = K1 * F               # 24 (k1, fc) pairs
NJB = NJ + 1              # 25-wide blocks: A(24) + bias
NA = EL * NJB             # 400 A/bias columns
NW = NA + NJ              # 424 matmul columns (f block is 24 wide)
LT = 128                  # l-tile size
NT = L // LT              # 16 l-tiles
UC = 4                    # l-tiles per u chunk
UCOLS = UC * LT + 2       # 514
UH = 2 * LT + 2           # 258: first chunk is split for an earlier start
NU = NT // UC             # 4 u chunks
PSW = 512                 # psum columns per sub-tile (bank-aligned)
NWARM = 6                 # PE clock-ramp matmuls before the real stream
WARMC = 512               # warm-up matmul column count
QT = 2                    # max l-tiles per group
# Pairs, with the last two tiles as singles (shortest end-of-stream
# drain chains; [15] additionally splits its f columns out so the fone
# ACT copy overlaps its A matmuls).
GROUPS = [[0, 1], [2, 3], [4, 5], [6, 7], [8, 9], [10, 11], [12, 13],
          [14], [15]]
# PSUM accumulation is order-independent; visit k in DMA-arrival order
# (w0 | w2 | w1) so the early DMA-gated tiles never idle the PE.  In
# pair groups the two sub-tiles interleave (a0,b0,a2,b2,a1,b1) so w2 is
# needed only by matmul 3 and w1 by matmul 5 of the group.  Together
# with NWARM=6 the PE has ZERO idle from first instruction to stream
# end, which reliably earns the early (~2.5us) utilization-cap unlock.
KORD = (0, 2, 1)

_OP_NAME = "SEG_MUL_CUMSUM_ANT"


def _ensure_custom_op():
    """Register the fused multiply+cumsum DVE op via the documented
    dve_ops extension point (idempotent)."""
    import concourse.dve_ops as dve_ops

    for op in dve_ops.OPS:
        if op.name == _OP_NAME:
            return op
    from concourse.dve_spec import AluOp, Spec, Src0, Src1, lower, scan
    from concourse.dve_spec import _has_src1
    from concourse.dve_uop import DveOpSpec

    def _ref(in0, in1, s0, s1, imm2):
        p, rest = in0.shape[0], int(np.prod(in0.shape[1:]))
        prod = (in0.astype(np.float32) * in1.astype(np.float32)).reshape(p, rest)
        return np.cumsum(prod, axis=1).reshape(in0.shape)

    spec = Spec(body=scan(AluOp.ADD, Src0 * Src1), reference=_ref)
    row = 1 + len(dve_ops.OPS)
    assert row < 0x20, "custom-DVE row field overflow"
    shas = {}
    for ver in ("v3", "v4"):
        u = lower(spec, ver=ver)
        shas[ver] = DveOpSpec(
            name=_OP_NAME, opcode=row, uops=u, rd1_en=_has_src1(spec)
        ).sha(ver)
    op = dve_ops.DveOp(_OP_NAME, spec, subdim=False, uops_sha=shas)
    dve_ops.OPS.append(op)
    dve_ops.CUSTOM_DVE_SPECS[op.name] = op.spec
    dve_ops._SUB_OPCODE_FOR_NAME[op.name] = row
    return op


def _build_program():
    import concourse.bass as bass
    import concourse.bacc as bacc
    import concourse.tile as tile
    from concourse import mybir

    # Fall back to a stock mult+reduce combine (~2.5us slower) if the
    # custom-op registration ever fails in the target environment.
    try:
        seg_op = _ensure_custom_op()
    except Exception:
        seg_op = None

    f32 = mybir.dt.float32
    bf16 = mybir.dt.bfloat16
    nc = bacc.Bacc("TRN2", target_bir_lowering=False, debug=False)

    u_dram = nc.dram_tensor("u_padt", [D, L + 2], bf16, kind="ExternalInput")
    w_dram = nc.dram_tensor("w_aug", [D, K2 * NW], bf16, kind="ExternalInput")
    # out[p, t*EL + e] = out_full[t*LT + p, e_local]
    o_dram = nc.dram_tensor("out", [LT, NT * EL], f32, kind="ExternalOutput")
    # keep-alive sink for the PE warm-up matmuls (ignored by the host)
    warm_dram = nc.dram_tensor("warm", [1, 1], bf16, kind="ExternalOutput")

    # Persistent output staging: 1KB/partition fp32, flushed by a single
    # post-TileContext DMA that drains under the walrus epilogue.
    o_sb = nc.alloc_sbuf_tensor("o_sb", [LT, NT, EL], f32)

    with tile.TileContext(nc) as tc:
        import contextlib

        with contextlib.ExitStack() as ctx:
            const_pool = ctx.enter_context(tc.tile_pool(name="const", bufs=1))
            psum_pool = ctx.enter_context(
                tc.tile_pool(name="psum", bufs=4, space="PSUM")
            )
            fonep = ctx.enter_context(tc.tile_pool(name="fone", bufs=4))
            cump = ctx.enter_context(tc.tile_pool(name="cum", bufs=4))

            # u chunks: the first 4 tiles use two 2-tile chunks (earlier
            # start + precise DMA deps); the rest use 4-tile chunks.
            u_sbs = []
            for g in range(NU):
                u_g = const_pool.tile([D, UCOLS], bf16, tag=f"u{g}", name=f"u{g}")
                u_sbs.append(u_g)
            ua = const_pool.tile([D, UH], bf16, name="ua")
            ub = const_pool.tile([D, UH], bf16, name="ub")
            w_sb = const_pool.tile([D, K2 * NW], bf16)

            def u_window(t, k):
                # [128 x 128] lhs window for tile t, shift k
                if t < 2:
                    return ua[:, t * LT + k : t * LT + k + LT]
                if t < 4:
                    return ub[:, (t - 2) * LT + k : (t - 2) * LT + k + LT]
                return u_sbs[t // UC][:, (t % UC) * LT + k : (t % UC) * LT + k + LT]

            def dma_u(g, eng):
                eng.dma_start(
                    out=u_sbs[g][:],
                    in_=u_dram[:, g * UC * LT : g * UC * LT + UCOLS],
                )

            def dma_w(k, eng):
                eng.dma_start(
                    out=w_sb[:, k * NW : (k + 1) * NW],
                    in_=w_dram[:, k * NW : (k + 1) * NW],
                )

            # DMA schedule: identical to the proven baseline (any PE-idle
            # gap before the tensor engine's utilization-cap unlock —
            # ~2.7us of continuous PE activity — postpones the unlock and
            # halves the whole stream, so the early phase must replicate
            # the tuned warm-up/DMA abutment exactly).
            #   sync:   w0 | w2 | u3;  scalar: ua | w1 | u1;  gpsimd: ub | u2
            dma_w(0, nc.sync)
            nc.scalar.dma_start(out=ua[:], in_=u_dram[:, 0:UH])
            nc.gpsimd.dma_start(out=ub[:], in_=u_dram[:, 2 * LT : 2 * LT + UH])
            dma_w(1, nc.scalar)
            dma_w(2, nc.sync)
            dma_u(2, nc.gpsimd)
            dma_u(1, nc.scalar)
            dma_u(3, nc.sync)

            # PE warm-up on the framework's pre-initialized bf16 const tile
            # via stride-0 APs: no memset dependency, so the DVFS ramp starts
            # the moment the PE enters the body.
            one_ap = nc.const_aps.aps[(bf16, 1.0)]
            warm_in0 = bass.AP(
                tensor=one_ap.tensor, offset=one_ap.offset,
                ap=[one_ap.ap[0], [0, LT]],
            )
            warm_in1 = bass.AP(
                tensor=one_ap.tensor, offset=one_ap.offset,
                ap=[one_ap.ap[0], [0, WARMC]],
            )
            warm_ps = psum_pool.tile([LT, QT, PSW], f32, tag="ps", name="warm_ps")
            for i in range(NWARM):
                nc.tensor.matmul(
                    warm_ps[:, 0, 0:WARMC],
                    warm_in0,
                    warm_in1,
                    start=(i == 0),
                    stop=(i == NWARM - 1),
                )
            warm_1 = bass.AP(
                tensor=one_ap.tensor, offset=one_ap.offset,
                ap=[[one_ap.ap[0][0], 1], [1, 1]],
            )
            nc.sync.dma_start(out=warm_dram[:], in_=warm_1)

            # rotating buffers: fone gets its 1.0 (bias) / 0.0 (pad) block
            # cols once; cum gets its seed column (global cumsum "-1" = 0).
            # Memsets run on the Vector engine (idle until the first scan);
            # gpsimd must keep its SW-DGE queue fed instead.
            fones, cums = [], []
            for b in range(4):
                fone = fonep.tile([LT, QT, NA], bf16, tag="fone", name=f"fone{b}")
                f4 = fone[:].rearrange("p q (e j) -> p q e j", j=NJB)
                nc.vector.memset(f4[:, :, :, NJ : NJB], 1.0)
                fones.append(fone)
                # width 1 + QT*NA + NJB: the trailing NJB cols are slack so
                # the boundary-view slices stay in bounds (never read).
                cum = cump.tile(
                    [LT, 1 + QT * NA + NJB], f32, tag="cum", name=f"cum{b}"
                )
                nc.vector.memset(cum[:, 0:1], 0.0)
                cums.append(cum)
            # The LAST PAIR group's chain binds the tail (T + fone-ACT 0.9
            # + scan 1.08); it alone uses the broadcast-f path: a 25-col
            # f block read by per-sub-tile scans through a stride-0 AP,
            # so its scans start ~0.8us earlier.  Per-sub-tile cumsums
            # need their own seed column.
            fblk6 = fonep.tile([LT, QT, NJB], bf16, tag="fblk6", name="fblk6")
            nc.vector.memset(fblk6[:, :, NJ:NJB], 1.0)
            cum6 = cump.tile(
                [LT, QT, 1 + NA + NJB], f32, tag="cum6", name="cum6"
            )
            nc.vector.memset(cum6[:, :, 0:1], 0.0)

            o_view = o_sb.ap().rearrange("p t e -> p t e")

            for g, tiles in enumerate(GROUPS):
                q = len(tiles)
                ps = psum_pool.tile([LT, q, PSW], f32, tag="ps", name="ps")
                last_group = q == 1
                if last_group:
                    # f/proj mini-groups for BOTH tiles first, then the A
                    # groups: the fone ACT copy overlaps the final ~1.1us
                    # of A matmuls instead of serializing after them.
                    for i, t in enumerate(tiles):
                        for j, k in enumerate(KORD):
                            nc.tensor.matmul(
                                ps[:, i, NA:NW],
                                u_window(t, k),
                                w_sb[:, k * NW + NA : k * NW + NW],
                                start=(j == 0),
                                stop=(j == K2 - 1),
                            )
                    for i, t in enumerate(tiles):
                        for j, k in enumerate(KORD):
                            nc.tensor.matmul(
                                ps[:, i, 0:NA],
                                u_window(t, k),
                                w_sb[:, k * NW : k * NW + NA],
                                start=(j == 0),
                                stop=(j == K2 - 1),
                            )
                else:
                    # (sub-tile, j) emission order: pairs interleave so the
                    # last-arriving w block (k=1) gates only matmul 5 of 6.
                    if q == 2:
                        order = [(0, 0), (1, 0), (0, 1), (1, 1), (0, 2), (1, 2)]
                    else:
                        order = [(0, j) for j in range(K2)]
                    for i, j in order:
                        k = KORD[j]
                        nc.tensor.matmul(
                            ps[:, i, 0:NW],
                            u_window(tiles[i], k),
                            w_sb[:, k * NW : (k + 1) * NW],
                            start=(j == 0),
                            stop=(j == K2 - 1),
                        )

                # fone = f block broadcast over the 16 e-blocks, in ONE
                # ACT copy straight from PSUM (bias/pad cols pre-set above).
                # The broadcast-f group (len-3) skips it entirely.
                fone = fones[g % 4]
                if not (seg_op is not None and g == len(GROUPS) - 3):
                    f4 = fone[:, 0:q, :].rearrange(
                        "p q (e j) -> p q e j", j=NJB
                    )
                    fps = ps[:, :, NA:NW]
                    fbc = bass.AP(
                        tensor=fps.tensor,
                        offset=fps.offset,
                        ap=[fps.ap[0], fps.ap[1], [0, EL], fps.ap[2]],
                    )
                    nc.scalar.copy(out=f4[:, :, :, 0:NJ], in_=fbc)

                s0 = tiles[0]
                cum = cums[g % 4]
                if seg_op is not None and g == len(GROUPS) - 3:
                    # last pair group: broadcast-f path (see fblk6 above).
                    for i in range(q):
                        nc.scalar.copy(
                            out=fblk6[:, i, 0:NJ], in_=ps[:, i, NA:NW]
                        )
                        fb = fblk6[:, i, :]
                        in1b = bass.AP(
                            tensor=fb.tensor,
                            offset=fb.offset,
                            ap=[fb.ap[0], [0, EL], fb.ap[1]],
                        )
                        nc.vector._custom_dve(
                            seg_op,
                            out=cum6[:, i, 1 : 1 + NA].rearrange(
                                "p (e j) -> p e j", j=NJB
                            ),
                            in0=ps[:, i, 0:NA].rearrange(
                                "p (e j) -> p e j", j=NJB
                            ),
                            in1=in1b,
                        )
                    cur = cum6[:, 0:q, NJB : NJB + NA].rearrange(
                        "p q (e j) -> p q e j", e=EL, j=NJB
                    )[:, :, :, 0:1]
                    prev = cum6[:, 0:q, 0:NA].rearrange(
                        "p q (e j) -> p q e j", e=EL, j=NJB
                    )[:, :, :, 0:1]
                    nc.gpsimd.tensor_tensor(
                        out=o_view[:, s0 : s0 + q, :],
                        in0=cur,
                        in1=prev,
                        op=mybir.AluOpType.subtract,
                    )
                elif seg_op is not None:
                    # fused multiply+cumsum straight from PSUM (fp32 A,
                    # bf16 f)
                    cview = cum[:, 1 : 1 + q * NA].rearrange(
                        "p (q n) -> p q n", n=NA
                    )
                    nc.vector._custom_dve(
                        seg_op,
                        out=cview,
                        in0=ps[:, :, 0:NA],
                        in1=fone[:, 0:q, :],
                    )
                    # segment sums = boundary differences of the global
                    # cumsum: out[q,e] = cum[q*NA+e*NJB+NJB] -
                    # cum[q*NA+e*NJB] (both views pick col j=0 of each
                    # 25-block; cum[0] is the memset seed).
                    cur = cum[:, NJB : NJB + q * NA].rearrange(
                        "p (q e j) -> p q e j", e=EL, j=NJB
                    )[:, :, :, 0:1]
                    prev = cum[:, 0 : q * NA].rearrange(
                        "p (q e j) -> p q e j", e=EL, j=NJB
                    )[:, :, :, 0:1]
                    # The boundary-difference subtract is tiny (q*16 elems
                    # per partition); running it on the otherwise-idle
                    # GpSimd engine keeps the Vector engine's per-group
                    # cost at ~the PE group period, so scans never back up.
                    nc.gpsimd.tensor_tensor(
                        out=o_view[:, s0 : s0 + q, :],
                        in0=cur,
                        in1=prev,
                        op=mybir.AluOpType.subtract,
                    )
                else:
                    # stock fallback: TT multiply from PSUM + 1x reduce
                    prod = cum[:, 0 : q * NA].rearrange(
                        "p (q n) -> p q n", n=NA
                    )
                    nc.vector.tensor_tensor(
                        out=prod,
                        in0=ps[:, :, 0:NA],
                        in1=fone[:, 0:q, :],
                        op=mybir.AluOpType.mult,
                    )
                    nc.vector.reduce_sum(
                        out=o_view[:, s0 : s0 + q, :],
                        in_=prod.rearrange(
                            "p q (e j) -> p q e j", j=NJB
                        ),
                        axis=mybir.AxisListType.X,
                    )

    # Single output flush AFTER the TileContext exit barrier (which waits
    # for all compute): the DMA's data movement drains under the walrus
    # epilogue's fixed semaphore wipe instead of serializing before it.
    # walrus codegen requires sync info on dynamic DMAs; attach a
    # completion inc on a dedicated sem that nothing waits on (the
    # epilogue's queue drain + full sem wipe handle the rest).
    o_sem = nc.alloc_semaphore("o_flush_sem")
    nc.sync.dma_start(out=o_dram[:], in_=o_sb.ap()).then_inc(o_sem, 16)

    nc.compile()
    return nc


def _prep_inputs(u, proj, conv_w, conv_b):
    """Host-side layout prep: reshuffle + bf16 rounding only."""
    u_padt = np.zeros((D, L + 2), BF16)
    u_padt[:, 1 : L + 1] = np.ascontiguousarray(u[0].T).astype(BF16)

    in_maps = []
    for c in range(NCORES):
        e0 = c * EL
        w_aug = np.zeros((K2, D, NW), np.float32)
        # conv weights: m = d*K2 + k2 (in_channel-major, tap-minor)
        cw = conv_w[e0 : e0 + EL].reshape(EL, K1, F, D, K2)
        wmain = cw.transpose(4, 3, 0, 1, 2).reshape(K2, D, EL, NJ)
        wa = w_aug[:, :, :NA].reshape(K2, D, EL, NJB)
        wa[:, :, :, :NJ] = wmain
        # bias at j = 24 of each 25-wide block (multiplied by the 1.0 slot)
        cb = conv_b[e0 : e0 + EL, 0, :, 0].reshape(EL, D, K2)
        wa[:, :, :, NJ] = cb.transpose(2, 1, 0)
        # proj columns: only in the k2 == k1 matmul
        for k in range(K2):
            w_aug[k, :, NA + k * F : NA + (k + 1) * F] = proj
        w_flat = w_aug.transpose(1, 0, 2).reshape(D, K2 * NW).astype(BF16)
        in_maps.append(
            {"u_padt": u_padt, "w_aug": np.ascontiguousarray(w_flat)}
        )
    return in_maps


_PROGRAM_CACHE = {}


def _gather_out(res_core):
    # o_dram [128, NT, EL] with l = t*LT + p
    arr = res_core["out"].reshape(LT, NT, EL)
    return arr.transpose(1, 0, 2).reshape(L, EL)


def kernel(
    u,
    kernel_params_feat_proj,
    kernel_params_conv_weights,
    kernel_params_conv_bias,
):
    from concourse.bass_utils import run_bass_kernel_spmd

    u = np.asarray(u, np.float32)
    proj = np.asarray(kernel_params_feat_proj, np.float32)
    conv_w = np.asarray(kernel_params_conv_weights, np.float32)
    conv_b = np.asarray(kernel_params_conv_bias, np.float32)

    if "nc" not in _PROGRAM_CACHE:
        _PROGRAM_CACHE["nc"] = _build_program()
    nc = _PROGRAM_CACHE["nc"]

    in_maps = _prep_inputs(u, proj, conv_w, conv_b)
    res = run_bass_kernel_spmd(nc, in_maps, list(range(NCORES)))

    out = np.empty((B, L, E), np.float32)
    for c in range(NCORES):
        out[0, :, c * EL : (c + 1) * EL] = _gather_out(res.results[c])
    return out
